# revision 3
# baseline (speedup 1.0000x reference)
"""Paged GQA chunked-prefill attention for 8 Trainium2 NeuronCores.

Problem (hardcoded): B=4 seqs x Q=256 new tokens, H=32 query heads, KVH=8 kv
heads (GQA group G=4), D=128 head dim, paged KV cache of 512 blocks x 16
tokens, per-seq lengths in seq_lens (clamped to >= Q), causal masking.

Sharding: tensor-parallel over heads. Core h gets kv head h and query heads
h*4..h*4+3; block_tables/seq_lens are resolved host-side while packing the
shards; the output is all-gathered host-side over the hidden dim.

Per-core device kernel (seq b, kv chunk c of 128 positions):
  S^T[kv,q] = K_c^T q  (f32r matmuls, q = (b,g,t) flattened to 1024 cols)
  S^T += causal mask   (identity-lhsT matmul accumulating into PSUM)
  U = exp(SCALE * S^T) (ScalarE, writes a float32r tile)
  lt[:,qs] += U_qs^T @ 1  (N=2 matmuls: denominator with q on partitions)
  O^T += V_c^T ... PSUM accumulation over chunks
  epilogue: rlt = 1/lt; O = transpose(O^T) per 128-slice; out = O * rlt.

Only positions < seq_len are ever touched (kv chunks beyond are skipped), so
compute scales with the actual sequence lengths.
"""
import math

import numpy as np

import concourse.mybir as mybir
import concourse.tile as tile
from concourse import bacc
from concourse.bass_utils import run_bass_kernel_spmd

B, Q, H, D = 4, 256, 32, 128
KVH = 8
G = H // KVH
BLOCK = 16
NB = 128
KV = NB * BLOCK
NUM_BLOCKS = B * NB
SCALE = 1.0 / math.sqrt(D)
N_CORES = 8
CHUNK = 128
QCOLS = G * Q  # 1024 q columns per sequence per core

F32 = mybir.dt.float32
F32R = mybir.dt.float32r
NEG = -1.0e9


def _plan(seq_lens):
    """Per-seq chunk counts, offsets, and boundary-chunk mask tiles."""
    L = np.maximum(np.asarray(seq_lens, dtype=np.int64), Q)
    cb = [int((int(Lb) + CHUNK - 1) // CHUNK) for Lb in L]
    offs = np.concatenate([[0], np.cumsum(cb)]).astype(int)
    masked = []  # list of (b, c, mask[128,256])
    t = np.arange(Q)
    p = np.arange(CHUNK)
    for b in range(B):
        Lb = int(L[b])
        for c in range(cb[b]):
            if c * CHUNK + CHUNK - 1 > Lb - Q:
                kvpos = c * CHUNK + p
                m = np.where(
                    kvpos[:, None] > (Lb - Q) + t[None, :], NEG, 0.0
                ).astype(np.float32)
                masked.append((b, c, m))
    return L, cb, offs, masked


def _build(seq_lens):
    L, cb, offs, masked = _plan(seq_lens)
    C = int(offs[-1])
    nmask = len(masked)
    mask_np = np.concatenate([m for _, _, m in masked], axis=1)  # [128, nm*256]
    mask_idx = {(b, c): i for i, (b, c, _) in enumerate(masked)}
    ident_np = np.eye(CHUNK, dtype=np.float32)
    ones_np = np.ones((CHUNK, 2), dtype=np.float32)

    nc = bacc.Bacc(
        "TRN2", target_bir_lowering=False, debug=False, num_devices=N_CORES
    )
    kt_d = nc.dram_tensor("kt", [D, C * CHUNK], F32R, kind="ExternalInput")
    v_d = nc.dram_tensor("v", [CHUNK, C * CHUNK], F32R, kind="ExternalInput")
    qt_d = nc.dram_tensor("qt", [D, B * QCOLS], F32R, kind="ExternalInput")
    out_d = nc.dram_tensor("out", [B, 8, CHUNK, D], F32, kind="ExternalOutput")
    mask_d = nc.inline_tensor(mask_np, name="mask_const")
    ident_d = nc.inline_tensor(ident_np, name="ident_const")
    ones_d = nc.inline_tensor(ones_np, name="ones_const")

    exp = mybir.ActivationFunctionType.Exp

    with tile.TileContext(nc) as tc:
        with (
            tc.tile_pool(name="sbin", bufs=1) as sbin,
            tc.tile_pool(name="sbu", bufs=3) as sbu,
            tc.tile_pool(name="sbe", bufs=2) as sbe,
            tc.tile_pool(name="ps_s", bufs=2, space="PSUM") as ps_s,
            tc.tile_pool(name="ps_o", bufs=1, space="PSUM") as ps_o,
            tc.tile_pool(name="ps_lt", bufs=1, space="PSUM") as ps_lt,
            tc.tile_pool(name="ps_t", bufs=1, space="PSUM") as ps_t,
        ):
            ident = sbin.tile([CHUNK, CHUNK], F32, tag="ident")
            nc.sync.dma_start(ident[:], ident_d.ap())
            identr = sbin.tile([CHUNK, CHUNK], F32R, tag="identr")
            nc.gpsimd.dma_start(identr[:], ident_d.ap())
            ones = sbin.tile([CHUNK, 2], F32R, tag="ones")
            nc.gpsimd.dma_start(ones[:], ones_d.ap())
            masks = sbin.tile([CHUNK, nmask * Q], F32R, tag="masks")
            nc.gpsimd.dma_start(masks[:], mask_d.ap())

            kt_b, v_b, qt_b = [], [], []
            for b in range(B):
                w = cb[b] * CHUNK
                kt = sbin.tile([D, w], F32R, tag=f"kt{b}")
                nc.sync.dma_start(
                    kt[:], kt_d.ap()[:, offs[b] * CHUNK : offs[b] * CHUNK + w]
                )
                vt = sbin.tile([CHUNK, w], F32R, tag=f"v{b}")
                nc.sync.dma_start(
                    vt[:], v_d.ap()[:, offs[b] * CHUNK : offs[b] * CHUNK + w]
                )
                qt = sbin.tile([D, QCOLS], F32R, tag=f"qt{b}")
                nc.sync.dma_start(
                    qt[:], qt_d.ap()[:, b * QCOLS : (b + 1) * QCOLS]
                )
                kt_b.append(kt)
                v_b.append(vt)
                qt_b.append(qt)

            for b in range(B):
                nchunks = cb[b]
                o_ps = ps_o.tile([D, QCOLS], F32, tag="o")
                lt_ps = ps_lt.tile([CHUNK, 16], F32, tag="lt")
                for c in range(nchunks):
                    first, last = c == 0, c == nchunks - 1
                    mi = mask_idx.get((b, c))
                    s_ps = ps_s.tile([CHUNK, QCOLS], F32, tag="s")
                    for n in range(2):
                        ncol = slice(n * 512, (n + 1) * 512)
                        nc.tensor.matmul(
                            s_ps[:, ncol],
                            kt_b[b][:, c * CHUNK : (c + 1) * CHUNK],
                            qt_b[b][:, ncol],
                            start=True,
                            stop=mi is None,
                        )
                    if mi is not None:
                        mb = (
                            masks[:, mi * Q : (mi + 1) * Q]
                            .unsqueeze(1)
                            .broadcast_to([CHUNK, 2, Q])
                        )
                        for n in range(2):
                            ncol = slice(n * 512, (n + 1) * 512)
                            nc.tensor.matmul(
                                s_ps[:, ncol],
                                identr[:],
                                mb,
                                start=False,
                                stop=True,
                            )
                    u = sbu.tile([CHUNK, QCOLS], F32R, tag="u")
                    nc.scalar.activation(u[:], s_ps[:], exp, scale=SCALE)
                    for qs in range(8):
                        nc.tensor.matmul(
                            lt_ps[:, qs * 2 : qs * 2 + 2],
                            u[:, qs * CHUNK : (qs + 1) * CHUNK],
                            ones[:, 0:2],
                            start=first and qs == 0,
                            stop=last and qs == 7,
                        )
                    for n in range(2):
                        ncol = slice(n * 512, (n + 1) * 512)
                        nc.tensor.matmul(
                            o_ps[:, ncol],
                            v_b[b][:, c * CHUNK : (c + 1) * CHUNK],
                            u[:, ncol],
                            start=first,
                            stop=last,
                        )
                rlt = sbe.tile([CHUNK, 16], F32, tag="rlt")
                nc.vector.reciprocal(rlt[:], lt_ps[:])
                ocp = sbe.tile([D, QCOLS], F32, tag="ocp")
                nc.vector.tensor_copy(ocp[:], o_ps[:])
                out_sb = sbe.tile([CHUNK, 8 * D], F32, tag="osb")
                for half in range(2):
                    ot_ps = ps_t.tile([CHUNK, 4 * D], F32, tag="ot")
                    for j in range(4):
                        qs = half * 4 + j
                        nc.tensor.matmul(
                            ot_ps[:, j * D : (j + 1) * D],
                            ocp[:, qs * CHUNK : (qs + 1) * CHUNK],
                            ident[:],
                            is_transpose=True,
                            start=j == 0,
                            stop=j == 3,
                        )
                    for j in range(4):
                        qs = half * 4 + j
                        nc.vector.tensor_scalar_mul(
                            out_sb[:, qs * D : (qs + 1) * D],
                            ot_ps[:, j * D : (j + 1) * D],
                            rlt[:, qs * 2 : qs * 2 + 1],
                        )
                nc.sync.dma_start(
                    out_d.ap()[b].transpose([1, 0, 2]),
                    out_sb[:].rearrange("p (a d) -> p a d", a=8),
                )

    nc.compile()
    return nc, L, cb, offs


def _pack_inputs(query, k_cache, v_cache, block_tables, L, cb, offs):
    """Gather the paged cache and pack per-core shards in device layouts."""
    C = int(offs[-1])
    k_lin = k_cache[block_tables].reshape(B, KV, KVH, D)
    v_lin = v_cache[block_tables].reshape(B, KV, KVH, D)
    kt_all = np.zeros((KVH, D, C * CHUNK), dtype=np.float32)
    v_all = np.zeros((KVH, CHUNK, C * CHUNK), dtype=np.float32)
    for b in range(B):
        Lb, w = int(L[b]), cb[b] * CHUNK
        kk = np.zeros((w, KVH, D), dtype=np.float32)
        kk[:Lb] = k_lin[b, :Lb]
        # [w, KVH, D] -> [KVH, D, w]
        kt_all[:, :, offs[b] * CHUNK : offs[b] * CHUNK + w] = kk.transpose(
            1, 2, 0
        )
        vv = np.zeros((w, KVH, D), dtype=np.float32)
        vv[:Lb] = v_lin[b, :Lb]
        # [cb, 128, KVH, D] -> [KVH, 128, cb, D] -> [KVH, 128, w]
        v_all[:, :, offs[b] * CHUNK : offs[b] * CHUNK + w] = (
            vv.reshape(cb[b], CHUNK, KVH, D)
            .transpose(2, 1, 0, 3)
            .reshape(KVH, CHUNK, w)
        )
    # query [B,Q,H,D] -> [KVH, D, B, G, Q] -> [KVH, D, B*G*Q]
    qt_all = (
        query.transpose(2, 3, 0, 1)
        .reshape(KVH, G, D, B, Q)
        .transpose(0, 2, 3, 1, 4)
        .reshape(KVH, D, B * QCOLS)
    )
    qt_all = np.ascontiguousarray(qt_all, dtype=np.float32)
    return [
        {
            "kt": np.ascontiguousarray(kt_all[h]),
            "v": np.ascontiguousarray(v_all[h]),
            "qt": qt_all[h],
        }
        for h in range(KVH)
    ]


def _unpack_outputs(results):
    """[B,8,128,D] per core (q=(g,t) on rows) -> [B*Q, H*D]."""
    out = np.empty((B * Q, H * D), dtype=np.float32)
    for h, res in enumerate(results):
        o = res["out"].reshape(B, QCOLS, D).reshape(B, G, Q, D)
        o = o.transpose(0, 2, 1, 3).reshape(B * Q, G * D)
        out[:, h * G * D : (h + 1) * G * D] = o
    return out


def kernel(query, k_cache, v_cache, block_tables, seq_lens):
    query = np.asarray(query, dtype=np.float32)
    k_cache = np.asarray(k_cache, dtype=np.float32)
    v_cache = np.asarray(v_cache, dtype=np.float32)
    block_tables = np.asarray(block_tables, dtype=np.int64)
    nc, L, cb, offs = _build(np.asarray(seq_lens))
    in_maps = _pack_inputs(query, k_cache, v_cache, block_tables, L, cb, offs)
    res = run_bass_kernel_spmd(nc, in_maps, core_ids=list(range(N_CORES)))
    return _unpack_outputs(res.results)


# revision 4
# speedup vs baseline: 1.2241x; 1.2241x over previous
"""Paged GQA chunked-prefill attention for 8 Trainium2 NeuronCores.

Problem (hardcoded): B=4 seqs x Q=256 new tokens, H=32 query heads, KVH=8 kv
heads (GQA group G=4), D=128 head dim, paged KV cache of 512 blocks x 16
tokens, per-seq lengths in seq_lens (clamped to >= Q), causal masking.

Sharding: tensor-parallel over heads. Core h gets kv head h and query heads
h*4..h*4+3; block_tables/seq_lens are resolved host-side while packing the
shards; the output is all-gathered host-side over the hidden dim.

Per-core device kernel (seq b, kv chunk c of 128 positions, q = (g,t) -> 1024
columns, processed in two 512-column halves n):
  S^T[kv,qh] = K_c^T q            (f32r matmul, full PE rate)
  S^T += causal mask              (identity-lhsT matmul into the same bank)
  U = exp(SCALE * S^T)            (ScalarE, PSUM->SBUF, float32r out)
  l[2,qh] += ones2^T @ U          (wide denominator matmul, q stays on free)
  O^T[d,qh] += V_c^T @ U          (PSUM accumulation over chunks)
Per-seq epilogue: l -> SBUF (ScalarE), PE-transpose l to [128,8] and O^T to
[q,d], rlt = 1/l (VectorE), out = O * rlt (tensor_scalar), DMA out.

Sequences are processed longest-first so the PE warms up on the big unmasked
run while the remaining DMAs and the mask constants stream in.
"""
import math

import numpy as np

import concourse.mybir as mybir
import concourse.tile as tile
from concourse import bacc
from concourse.bass_utils import run_bass_kernel_spmd

B, Q, H, D = 4, 256, 32, 128
KVH = 8
G = H // KVH
BLOCK = 16
NB = 128
KV = NB * BLOCK
NUM_BLOCKS = B * NB
SCALE = 1.0 / math.sqrt(D)
N_CORES = 8
CHUNK = 128
QCOLS = G * Q  # 1024 q columns per sequence per core
NHALF = 512

F32 = mybir.dt.float32
F32R = mybir.dt.float32r
NEG = -1.0e9


def _plan(seq_lens):
    """Per-seq chunk counts, offsets, and boundary-chunk mask tiles."""
    L = np.maximum(np.asarray(seq_lens, dtype=np.int64), Q)
    cb = [int((int(Lb) + CHUNK - 1) // CHUNK) for Lb in L]
    offs = np.concatenate([[0], np.cumsum(cb)]).astype(int)
    masked = []  # list of (b, c, mask[128,256])
    t = np.arange(Q)
    p = np.arange(CHUNK)
    for b in range(B):
        Lb = int(L[b])
        for c in range(cb[b]):
            if c * CHUNK + CHUNK - 1 > Lb - Q:
                kvpos = c * CHUNK + p
                m = np.where(
                    kvpos[:, None] > (Lb - Q) + t[None, :], NEG, 0.0
                ).astype(np.float32)
                masked.append((b, c, m))
    return L, cb, offs, masked


def _build(seq_lens):
    L, cb, offs, masked = _plan(seq_lens)
    C = int(offs[-1])
    nmask = len(masked)
    mask_np = np.concatenate([m for _, _, m in masked], axis=1)  # [128, nm*256]
    mask_idx = {(b, c): i for i, (b, c, _) in enumerate(masked)}
    ident_np = np.eye(CHUNK, dtype=np.float32)
    ones_np = np.ones((CHUNK, 2), dtype=np.float32)
    border = sorted(range(B), key=lambda b: -cb[b])  # longest first

    nc = bacc.Bacc(
        "TRN2", target_bir_lowering=False, debug=False, num_devices=N_CORES
    )
    kt_d = nc.dram_tensor("kt", [D, C * CHUNK], F32R, kind="ExternalInput")
    v_d = nc.dram_tensor("v", [CHUNK, C * CHUNK], F32R, kind="ExternalInput")
    qt_d = nc.dram_tensor("qt", [D, B * QCOLS], F32R, kind="ExternalInput")
    out_d = nc.dram_tensor("out", [B, 8, CHUNK, D], F32, kind="ExternalOutput")
    mask_d = nc.inline_tensor(mask_np, name="mask_const")
    ident_d = nc.inline_tensor(ident_np, name="ident_const")
    ones_d = nc.inline_tensor(ones_np, name="ones_const")

    exp = mybir.ActivationFunctionType.Exp

    with tile.TileContext(nc) as tc:
        with (
            tc.tile_pool(name="sbin", bufs=1) as sbin,
            tc.tile_pool(name="sbu", bufs=4) as sbu,
            tc.tile_pool(name="sbe", bufs=2) as sbe,
            tc.tile_pool(name="ps_s", bufs=3, space="PSUM") as ps_s,
            tc.tile_pool(name="ps_o", bufs=1, space="PSUM") as ps_o,
            tc.tile_pool(name="ps_l", bufs=1, space="PSUM") as ps_l,
            tc.tile_pool(name="ps_tp", bufs=1, space="PSUM") as ps_tp,
        ):
            # Critical-path DMAs first: K chunk 0 / first q half of the
            # first (longest) sequence, so the PE starts ~10us earlier.
            b0 = border[0]
            kt_t = [None] * B
            qt_t = [None] * B
            v_t = [None] * B
            w0 = cb[b0] * CHUNK
            kt_first = sbin.tile([D, w0], F32R, tag=f"kt{b0}")
            nc.sync.dma_start(
                kt_first[:, 0:CHUNK],
                kt_d.ap()[:, offs[b0] * CHUNK : offs[b0] * CHUNK + CHUNK],
            )
            qt_first = sbin.tile([D, QCOLS], F32R, tag=f"qt{b0}")
            nc.sync.dma_start(
                qt_first[:, 0:NHALF],
                qt_d.ap()[:, b0 * QCOLS : b0 * QCOLS + NHALF],
            )
            nc.sync.dma_start(
                qt_first[:, NHALF:QCOLS],
                qt_d.ap()[:, b0 * QCOLS + NHALF : (b0 + 1) * QCOLS],
            )
            nc.sync.dma_start(
                kt_first[:, CHUNK:w0],
                kt_d.ap()[:, offs[b0] * CHUNK + CHUNK : offs[b0] * CHUNK + w0],
            )
            kt_t[b0] = kt_first
            qt_t[b0] = qt_first

            ones = sbin.tile([CHUNK, 2], F32R, tag="ones")
            nc.gpsimd.dma_start(ones[:], ones_d.ap())
            identr = sbin.tile([CHUNK, CHUNK], F32R, tag="identr")
            nc.gpsimd.dma_start(identr[:], ident_d.ap())
            masks = sbin.tile([CHUNK, nmask * Q], F32R, tag="masks")
            nc.gpsimd.dma_start(masks[:], mask_d.ap())
            ident = sbin.tile([CHUNK, CHUNK], F32, tag="ident")
            nc.sync.dma_start(ident[:], ident_d.ap())

            for b in border:
                w = cb[b] * CHUNK
                if kt_t[b] is None:
                    kt = sbin.tile([D, w], F32R, tag=f"kt{b}")
                    nc.sync.dma_start(
                        kt[:],
                        kt_d.ap()[:, offs[b] * CHUNK : offs[b] * CHUNK + w],
                    )
                    kt_t[b] = kt
                    qt = sbin.tile([D, QCOLS], F32R, tag=f"qt{b}")
                    nc.sync.dma_start(
                        qt[:], qt_d.ap()[:, b * QCOLS : (b + 1) * QCOLS]
                    )
                    qt_t[b] = qt
                vt = sbin.tile([CHUNK, w], F32R, tag=f"v{b}")
                nc.sync.dma_start(
                    vt[:], v_d.ap()[:, offs[b] * CHUNK : offs[b] * CHUNK + w]
                )
                v_t[b] = vt

            for b in border:
                nchunks = cb[b]
                o_ps = ps_o.tile([D, QCOLS], F32, tag="o")
                l_ps = ps_l.tile([2, QCOLS], F32, tag="l")
                for c in range(nchunks):
                    first, last = c == 0, c == nchunks - 1
                    mi = mask_idx.get((b, c))
                    for n in range(2):
                        ncol = slice(n * NHALF, (n + 1) * NHALF)
                        s_ps = ps_s.tile([CHUNK, NHALF], F32, tag="s")
                        nc.tensor.matmul(
                            s_ps[:],
                            kt_t[b][:, c * CHUNK : (c + 1) * CHUNK],
                            qt_t[b][:, ncol],
                            start=True,
                            stop=mi is None,
                        )
                        if mi is not None:
                            mb = (
                                masks[:, mi * Q : (mi + 1) * Q]
                                .unsqueeze(1)
                                .broadcast_to([CHUNK, 2, Q])
                            )
                            nc.tensor.matmul(
                                s_ps[:], identr[:], mb, start=False, stop=True
                            )
                        u = sbu.tile([CHUNK, NHALF], F32R, tag="u")
                        nc.scalar.activation(u[:], s_ps[:], exp, scale=SCALE)
                        nc.tensor.matmul(
                            l_ps[:, ncol],
                            ones[:, 0:2],
                            u[:],
                            start=first,
                            stop=last,
                        )
                        nc.tensor.matmul(
                            o_ps[:, ncol],
                            v_t[b][:, c * CHUNK : (c + 1) * CHUNK],
                            u[:],
                            start=first,
                            stop=last,
                        )
                # epilogue: denominators to q-on-partition layout
                l_sb = sbe.tile([2, QCOLS], F32, tag="lsb")
                nc.scalar.copy(l_sb[:], l_ps[:])
                lt_ps = ps_tp.tile([CHUNK, 8], F32, tag="tp")
                for qs in range(8):
                    nc.tensor.matmul(
                        lt_ps[:, qs : qs + 1],
                        l_sb[0:1, qs * CHUNK : (qs + 1) * CHUNK],
                        ident[0:1, 0:1],
                        is_transpose=True,
                        start=qs == 0,
                        stop=qs == 7,
                    )
                rlt = sbe.tile([CHUNK, 8], F32, tag="rlt")
                nc.vector.reciprocal(rlt[:], lt_ps[:])
                ocp = sbe.tile([D, QCOLS], F32, tag="ocp")
                nc.vector.tensor_copy(ocp[:], o_ps[:])
                out_sb = sbe.tile([CHUNK, 8 * D], F32, tag="osb")
                for half in range(2):
                    ot_ps = ps_tp.tile([CHUNK, 4 * D], F32, tag="tp")
                    for j in range(4):
                        qs = half * 4 + j
                        nc.tensor.matmul(
                            ot_ps[:, j * D : (j + 1) * D],
                            ocp[:, qs * CHUNK : (qs + 1) * CHUNK],
                            ident[:],
                            is_transpose=True,
                            start=j == 0,
                            stop=j == 3,
                        )
                    for j in range(4):
                        qs = half * 4 + j
                        nc.vector.tensor_scalar_mul(
                            out_sb[:, qs * D : (qs + 1) * D],
                            ot_ps[:, j * D : (j + 1) * D],
                            rlt[:, qs : qs + 1],
                        )
                nc.sync.dma_start(
                    out_d.ap()[b].transpose([1, 0, 2]),
                    out_sb[:].rearrange("p (a d) -> p a d", a=8),
                )

    nc.compile()
    return nc, L, cb, offs


def _pack_inputs(query, k_cache, v_cache, block_tables, L, cb, offs):
    """Gather the paged cache and pack per-core shards in device layouts."""
    C = int(offs[-1])
    k_lin = k_cache[block_tables].reshape(B, KV, KVH, D)
    v_lin = v_cache[block_tables].reshape(B, KV, KVH, D)
    kt_all = np.zeros((KVH, D, C * CHUNK), dtype=np.float32)
    v_all = np.zeros((KVH, CHUNK, C * CHUNK), dtype=np.float32)
    for b in range(B):
        Lb, w = int(L[b]), cb[b] * CHUNK
        kk = np.zeros((w, KVH, D), dtype=np.float32)
        kk[:Lb] = k_lin[b, :Lb]
        # [w, KVH, D] -> [KVH, D, w]
        kt_all[:, :, offs[b] * CHUNK : offs[b] * CHUNK + w] = kk.transpose(
            1, 2, 0
        )
        vv = np.zeros((w, KVH, D), dtype=np.float32)
        vv[:Lb] = v_lin[b, :Lb]
        # [cb, 128, KVH, D] -> [KVH, 128, cb, D] -> [KVH, 128, w]
        v_all[:, :, offs[b] * CHUNK : offs[b] * CHUNK + w] = (
            vv.reshape(cb[b], CHUNK, KVH, D)
            .transpose(2, 1, 0, 3)
            .reshape(KVH, CHUNK, w)
        )
    # query [B,Q,H,D] -> [KVH, D, B, G, Q] -> [KVH, D, B*G*Q]
    qt_all = (
        query.transpose(2, 3, 0, 1)
        .reshape(KVH, G, D, B, Q)
        .transpose(0, 2, 3, 1, 4)
        .reshape(KVH, D, B * QCOLS)
    )
    qt_all = np.ascontiguousarray(qt_all, dtype=np.float32)
    return [
        {
            "kt": np.ascontiguousarray(kt_all[h]),
            "v": np.ascontiguousarray(v_all[h]),
            "qt": qt_all[h],
        }
        for h in range(KVH)
    ]


def _unpack_outputs(results):
    """[B,8,128,D] per core (q=(g,t) on rows) -> [B*Q, H*D]."""
    out = np.empty((B * Q, H * D), dtype=np.float32)
    for h, res in enumerate(results):
        o = res["out"].reshape(B, QCOLS, D).reshape(B, G, Q, D)
        o = o.transpose(0, 2, 1, 3).reshape(B * Q, G * D)
        out[:, h * G * D : (h + 1) * G * D] = o
    return out


def kernel(query, k_cache, v_cache, block_tables, seq_lens):
    query = np.asarray(query, dtype=np.float32)
    k_cache = np.asarray(k_cache, dtype=np.float32)
    v_cache = np.asarray(v_cache, dtype=np.float32)
    block_tables = np.asarray(block_tables, dtype=np.int64)
    nc, L, cb, offs = _build(np.asarray(seq_lens))
    in_maps = _pack_inputs(query, k_cache, v_cache, block_tables, L, cb, offs)
    res = run_bass_kernel_spmd(nc, in_maps, core_ids=list(range(N_CORES)))
    return _unpack_outputs(res.results)
